# revision 29
# baseline (speedup 1.0000x reference)
"""ConvSelfAttention Trainium2 kernel.

Reference computation (per batch b, with x flattened to [C=128, N=4096]):
    q = wq @ x + bq        [64, N]   (scaled by 1/sqrt(128), folded into wq/bq)
    k = wk @ x + bk        [64, N]
    v = wv @ x + bv        [64, N]
    s[i,j] = sum_o q[o,i] k[o,j]
    p = softmax_j(s)
    out[o,i] = sum_j v[o,j] p[i,j]
    y = gamma * (wo @ out + bo) + x

Mapping (one batch per NeuronCore, 8 cores):
  - scores are built TRANSPOSED: sT[j,i] = sum_o k[o,j] q[o,i], j-tile (128) on
    partitions, i-block (512) on free dim; QK has K=64 so q/k are DUPLICATED in
    both partition halves and consecutive j-tiles run CONCURRENTLY in the PE
    array (row tile_position via partition offsets).
  - softmax exp is SPLIT between ScalarE (true table exp) and a CUSTOM DVE
    micro-op (EXP_POLY_SQ: (1 + t(e1 + t(e2 + t)))^2, t = c*s — a minimax fit
    on the score range bounded at build time from the actual inputs).  Both
    write p tiles in fp8e4 into a contiguous SBUF ring.
  - PV runs as fp8 DoubleRow matmuls: stationary [vT(2r) | vT(2r+1)] pairs
    (pad stride 80), moving pT pairs [128, 2, 512] — contraction over 256 j
    per instruction at 2 MACs/cell/cycle.  A ones column (col 64) makes PSUM
    row 64 the softmax denominator Z_i.
  - epilogue: rden = 1/Z (DVE), DMA partition-broadcast of rden to 64 rows,
    normalize+cast (DVE), output projection (PE), y = ps + gbo + x (DVE stt).
    v's bias folds into gbo = gamma*(wo@bv+bo) since sum_j p = 1; gamma folds
    into wo/bo on the host.
"""

import sys

import numpy as np

try:
    import concourse  # noqa: F401
except ImportError:  # pragma: no cover
    sys.path.insert(0, "/opt/trn_rl_repo")

import ml_dtypes

B, C, CO, N = 8, 128, 64, 4096
W = H = 64
NCORES = 8
IBLK = 512          # query columns per i-block
NJT = N // 128      # 32 j-tiles of 128 keys
NIB = N // IBLK     # 8 i-blocks
JGRP = 3            # j-tiles per exp group (3 PSUM banks)
VSTRIDE = 128       # vT tile stride: cols 0:64 = v, 64:128 = ones (Z rows)

_CACHE = {}


def _split_multiwaits(nc):
    """Workaround for the pinned walrus: it accepts at most ONE semaphore wait
    per instruction (setupSyncWait: "Too many sync wait commands").  Hoist all
    but the last wait of any instruction onto single-wait NoOps inserted just
    before it in the same engine's stream — semantically identical (the engine
    blocks on each wait in turn before issuing the instruction)."""
    from concourse import mybir

    nsplit = 0
    for fn in nc.m.functions:
        for bb in fn.blocks:
            out = []
            for inst in bb.instructions:
                si = inst.sync_info
                if si is not None and si.on_wait is not None and len(si.on_wait) > 1:
                    waits = list(si.on_wait)
                    for i, w in enumerate(waits[:-1]):
                        out.append(mybir.InstNoOp(
                            name=f"{inst.name}-sw{i}",
                            engine=inst.engine,
                            sync_info=mybir.SyncInfo(on_wait=[w], on_update=[]),
                            bass_nofuse=True,
                        ))
                        nsplit += 1
                    si.on_wait = [waits[-1]]
                    inst.sync_info = si
                out.append(inst)
            bb.instructions = out
    return nsplit


def register_exp_poly():
    """Custom DVE op: exp(s) ~ (1 + t(C1 + t(C2 + t)))^2 with t = C0*s.
    7 ALU stages, single-src (Src1-broadcast hangs on HW here); coefficients
    arrive per-call via s0/s1/imm2, so one registration serves any fit."""
    from concourse import dve_ops
    from concourse.dve_spec import Spec, Src0, C0, C1, C2, One, sq, lower, _has_src1
    from concourse.dve_uop import DveOpSpec

    name = "EXP_POLY_SQ"
    for op in dve_ops.OPS:
        if op.name == name:
            return op
    t = C0 * Src0
    spec = Spec(
        body=sq(One + t * (C1 + t * (C2 + t))),
        reference=lambda in0, in1, s0, s1, imm2: np.square(
            1.0 + (s0 * in0) * (s1 + (s0 * in0) * (imm2 + (s0 * in0)))
        ),
    )
    dve_ops._SUB_OPCODE_FOR_NAME[name] = dve_ops._CUSTOM_DVE_ROW_BASE + len(
        dve_ops.OPS
    )
    shas = {}
    for ver in ("v3", "v4"):
        try:
            s = DveOpSpec(
                name=name,
                opcode=dve_ops.get_dve_sub_opcode(name),
                uops=lower(spec, ver=ver),
                rd1_en=_has_src1(spec),
            )
            shas[ver] = s.sha(ver)
        except Exception:  # v4 optional; only v3 (TRN2) is required here
            pass
    op = dve_ops.DveOp(name, spec, subdim=False, uops_sha=shas)
    dve_ops.OPS.append(op)
    dve_ops.CUSTOM_DVE_SPECS[name] = spec
    return op


def fit_exp_poly(R):
    """Fit (c, e1, e2) of EXP_POLY_SQ to exp on [-R, R] (max rel err)."""
    from scipy.optimize import minimize

    s = np.linspace(-R, R, 2001)
    tgt = np.exp(s)

    def err(params):
        c, e1, e2 = params
        t = c * s
        p = (1 + t * (e1 + t * (e2 + t))) ** 2
        return np.max(np.abs(p - tgt) / tgt)

    a = np.polyfit(s, np.exp(s / 2), 3)[::-1]
    a = a / a[0]
    c0 = float(np.cbrt(a[3]))
    best = None
    for sc in (1.0, 0.95, 1.05):
        r = minimize(err, [c0 * sc, a[1] / c0, a[2] / c0**2],
                     method="Nelder-Mead",
                     options=dict(maxiter=3000, xatol=1e-12, fatol=1e-14))
        if best is None or r.fun < best.fun:
            best = r
    return [float(v) for v in best.x], float(best.fun)


_FORCE = None  # (use_dve_exp, fast_recip) override for debugging


def build_nc(coef, act_frac=0.6165, use_dve_exp=True, fast_recip=True,
             warm_mm=2, debug=False):
    from concourse import mybir
    import concourse.bass as bass
    import concourse.tile as tile

    f32 = mybir.dt.float32
    bf16 = mybir.dt.bfloat16
    fp8 = mybir.dt.float8e4
    Alu = mybir.AluOpType
    Act = mybir.ActivationFunctionType

    exp_op = register_exp_poly()
    pc, pe1, pe2 = coef

    nc = bass.Bass()

    x_d = nc.dram_tensor("x", [C, N], f32, kind="ExternalInput")
    xb_d = nc.dram_tensor("xb", [C, N], bf16, kind="ExternalInput")  # host cast
    # packed bf16 weights: [wqT(128) | wkT(128) | wvT(64) | woT(128, rows 0:64)]
    wpack_d = nc.dram_tensor("wpack", [C, 448], bf16, kind="ExternalInput")
    # packed f32 scalars: [bq | bk | gbo]
    bpack_d = nc.dram_tensor("bpack", [C, 3], f32, kind="ExternalInput")
    y_d = nc.dram_tensor("y", [C, N], f32, kind="ExternalOutput")

    with tile.TileContext(nc) as tc:
        with (
            tc.tile_pool(name="consts", bufs=1) as consts,
            tc.tile_pool(name="big", bufs=1) as big,
            tc.tile_pool(name="epi", bufs=2) as epi,
        ):
            # ---- x chunk 0 DMA first (critical path), then packed weights ----
            x_sb = big.tile([C, N], f32)
            x_bf = big.tile([C, N], bf16)
            nc.sync.dma_start(x_bf[:, 0:512], xb_d[:, 0:512])

            wpack = consts.tile([C, 448], bf16)
            nc.gpsimd.dma_start(wpack, wpack_d[:, :])
            bpack = consts.tile([C, 3], f32)
            nc.gpsimd.dma_start(bpack, bpack_d[:, :])
            wqT = wpack[:, 0:128]
            wkT = wpack[:, 128:256]
            wvT = wpack[:, 256:320]
            woT = wpack[0:CO, 320:448]
            bq_s = bpack[:, 0:1]
            bk_s = bpack[:, 1:2]
            gbo = bpack[:, 2:3]

            # warm the exp table set (~2.7us ACT_TABLE_LOAD) during the ramp
            warm = consts.tile([C, 1], f32)
            nc.vector.memset(warm, 0.0)
            nc.scalar.activation(warm, warm, Act.Exp)



            # ---- x load + cast + projections, pipelined in 512-col chunks ----
            q_sb = big.tile([C, N], bf16)
            k_sb = big.tile([C, N], bf16)
            # vT tiles [128 j, 64 v | 64 ones] in fp8 (stride 128): the ones
            # columns make DoubleRow PV emit Z replicated on PSUM rows 64:128,
            # so no partition-broadcast of 1/Z is ever needed.
            vpad = big.tile([C, NJT * VSTRIDE], fp8)
            vpad3 = vpad.rearrange("p (t e) -> p t e", e=VSTRIDE)
            nc.gpsimd.memset(vpad3[:, :, CO:VSTRIDE], 1.0)  # ones columns

            with tc.tile_pool(name="setup_ps", bufs=4, space="PSUM") as setup_ps:
                for t in range(N // 512):
                    sl = slice(t * 512, (t + 1) * 512)
                    if t > 0:  # chunk 0 DMA already issued above
                        nc.sync.dma_start(x_bf[:, sl], xb_d[:, sl])
                    nc.sync.dma_start(x_sb[:, sl], x_d[:, sl])
                    ps_q = setup_ps.tile([C, 512], f32, tag="proj")
                    nc.tensor.matmul(ps_q, lhsT=wqT, rhs=x_bf[:, sl],
                                     start=True, stop=True)
                    nc.vector.tensor_scalar_add(q_sb[:, sl], ps_q, bq_s)
                    ps_k = setup_ps.tile([C, 512], f32, tag="proj")
                    nc.tensor.matmul(ps_k, lhsT=wkT, rhs=x_bf[:, sl],
                                     start=True, stop=True)
                    nc.scalar.activation(k_sb[:, sl], ps_k, Act.Identity,
                                         bias=bk_s)
                    ps_v = setup_ps.tile([C, 256], f32, tag="vt")
                    for tt in range(4):
                        nt = t * 4 + tt
                        nc.tensor.matmul(
                            ps_v[:, tt * CO:(tt + 1) * CO],
                            lhsT=x_bf[:, nt * 128:(nt + 1) * 128],
                            rhs=wvT,
                            start=True, stop=True,
                        )
                    nc.vector.tensor_copy(
                        vpad3[:, t * 4:(t + 1) * 4, 0:CO],
                        ps_v.rearrange("p (t e) -> p t e", e=CO),
                    )

            # fp8 pT ring: one i-block of 32 tiles x 512 cols, pairs contiguous
            ring = big.tile([C, NJT * 512], fp8)

            jgroups = [list(range(s, min(s + JGRP, NJT)))
                       for s in range(0, NJT, JGRP)]

            with (
                tc.tile_pool(name="qk_ps", bufs=2, space="PSUM") as qk_ps_pool,
                tc.tile_pool(name="pv_ps", bufs=1, space="PSUM") as pv_ps_pool,
                tc.tile_pool(name="oc_ps", bufs=1, space="PSUM") as oc_ps_pool,
            ):
                # the output-projection tail of i-block ib is deferred into
                # i-block ib+1's instruction stream: the PE's in-order queue
                # would otherwise stall ~2us per i-block behind the DVE
                # normalize chain (and the idle gap re-throttles PE-HAM).
                epi_tail = None

                warm_sb = consts.tile([CO, 64], bf16)
                nc.vector.memset(warm_sb, 0.0)
                ps_oc = oc_ps_pool.tile([C, IBLK], f32, tag="oc")

                def warm(n):
                    # tiny dummy matmuls into the oc bank (ordering keeps
                    # them clear of the projection->y2 window): PE-HAM
                    # re-throttles the array to 1.2 GHz unless the PE stream
                    # stays busy, so bridge the exp-paced gaps between groups.
                    for _ in range(n):
                        nc.tensor.matmul(ps_oc[0:CO, 0:64], lhsT=warm_sb,
                                         rhs=warm_sb, start=True, stop=True)

                for ib in range(NIB):
                    isl = slice(ib * IBLK, (ib + 1) * IBLK)
                    ps_pv = pv_ps_pool.tile([C, 512], f32, tag="pv")
                    next_pair = 0

                    def emit_pv(upto_pairs, ps_pv=ps_pv):
                        nonlocal next_pair
                        for r in range(next_pair, upto_pairs):
                            nc.tensor.matmul(
                                ps_pv,
                                lhsT=vpad3[:, 2 * r:2 * r + 2, :],
                                rhs=ring.rearrange(
                                    "p (t e) -> p t e", e=512
                                )[:, 2 * r:2 * r + 2, :],
                                start=(r == 0), stop=(r == NJT // 2 - 1),
                                perf_mode=mybir.MatmulPerfMode.DoubleRow,
                            )
                        next_pair = upto_pairs

                    done_tiles = 0
                    for g, jts in enumerate(jgroups):
                        glen = len(jts)
                        gw = glen * 512
                        ps_qk = qk_ps_pool.tile([128, JGRP * 512], f32)
                        for idx, jt in enumerate(jts):
                            half = jt % 2  # alternate row halves -> PE pairs
                            hsl = slice(half * CO, half * CO + CO)
                            nc.tensor.matmul(
                                ps_qk[:, idx * 512:(idx + 1) * 512],
                                lhsT=k_sb[hsl, jt * 128:(jt + 1) * 128],
                                rhs=q_sb[hsl, isl],
                                start=True, stop=True,
                            )
                        # exp: ACT takes [0:xa], custom-DVE poly takes [xa:gw]
                        off = jts[0] * 512
                        xa = int(round(act_frac * gw)) if use_dve_exp else gw
                        nc.scalar.activation(ring[:, off:off + xa],
                                             ps_qk[:, 0:xa], Act.Exp)
                        if xa < gw:
                            nc.vector._custom_dve(
                                exp_op,
                                out=ring[:, off + xa:off + gw],
                                in0=ps_qk[:, xa:gw],
                                s0=pc, s1=pe1, imm2=pe2,
                            )
                        # PV lags one group so the PE never waits on exp
                        if g >= 1:
                            emit_pv(done_tiles // 2)
                        done_tiles += glen
                        warm(1)
                        if g == 0 and epi_tail is not None:
                            epi_tail()
                            epi_tail = None
                    emit_pv(NJT // 2)

                    # ---- epilogue: ps_pv rows 64:128 hold Z replicated.
                    # Evacuate Z to partition 0:64 (native copy handles the
                    # partition offset; custom-DVE ops only work at offset 0),
                    # take 1/Z, normalize+cast, project.
                    zsb = epi.tile([CO, 512], f32, tag="zsb")
                    nc.vector.tensor_copy(zsb, ps_pv[CO:C, :])
                    rz = epi.tile([CO, 512], f32, tag="rz")
                    nc.vector.reciprocal_approx_fast(rz, zsb)
                    out_sb = epi.tile([CO, IBLK], bf16, tag="out")
                    nc.vector.tensor_tensor(out_sb, ps_pv[0:CO, :], rz,
                                            Alu.mult)

                    def epi_tail(isl=isl, out_sb=out_sb):
                        nc.tensor.matmul(ps_oc, lhsT=woT, rhs=out_sb,
                                         start=True, stop=True)
                        y2 = epi.tile([C, IBLK], f32, tag="y2")
                        nc.vector.scalar_tensor_tensor(
                            out=y2, in0=ps_oc, scalar=gbo, in1=x_sb[:, isl],
                            op0=Alu.add, op1=Alu.add,
                        )
                        nc.sync.dma_start(y_d[:, isl], y2)
                epi_tail()

    from concourse.library_overlay import lower_extended_insts
    lower_extended_insts(nc)
    _split_multiwaits(nc)
    return nc


def host_prep(inputs):
    """Fold scales/transposes on the host; returns the 8 per-core input maps
    and the exp-poly fit (R from a Cauchy-Schwarz bound on the scores)."""
    x = np.ascontiguousarray(np.asarray(inputs["x"], dtype=np.float32))
    wq = np.asarray(inputs["wq"], dtype=np.float32)
    bq = np.asarray(inputs["bq"], dtype=np.float32)
    wk = np.asarray(inputs["wk"], dtype=np.float32)
    bk = np.asarray(inputs["bk"], dtype=np.float32)
    wv = np.asarray(inputs["wv"], dtype=np.float32)
    bv = np.asarray(inputs["bv"], dtype=np.float32)
    wo = np.asarray(inputs["wo"], dtype=np.float32)
    bo = np.asarray(inputs["bo"], dtype=np.float32)
    gamma = float(np.asarray(inputs["gamma"]).reshape(-1)[0])

    s = 1.0 / np.sqrt(np.float32(C))
    bf = ml_dtypes.bfloat16
    wqTs = wq.T * s                                                    # [128,64]
    wqT = np.concatenate([wqTs, wqTs], axis=1)                         # [128,128]
    wkT = np.concatenate([wk.T, wk.T], axis=1)                         # [128,128]
    wvT = wv.T                                                         # [128,64]
    woT_pad = np.zeros((C, C), np.float32)
    woT_pad[:CO, :] = gamma * wo.T                                     # rows 0:64
    wpack = np.concatenate([wqT, wkT, wvT, woT_pad], axis=1).astype(bf)
    bq_s = np.concatenate([bq * s, bq * s])
    bk_s = np.concatenate([bk, bk])
    gbo = gamma * (wo @ bv + bo)
    bpack = np.stack([bq_s, bk_s, gbo], axis=1).astype(np.float32)     # [128,3]

    xb = x.reshape(B, C, N)
    # exact score range: max |q_i . k_j| / sqrt(C) over all pairs and batches
    # (~17 GFLOP of sgemm on the host, well under a second)
    qall = np.einsum("oc,bcn->bon", wq, xb) + bq[None, :, None]
    kall = np.einsum("oc,bcn->bon", wk, xb) + bk[None, :, None]
    rbound = 0.0
    for b in range(B):
        sb = qall[b].T @ kall[b]
        rbound = max(rbound, float(np.abs(sb).max()))
    rbound /= float(np.sqrt(C))

    in_maps = []
    for b in range(B):
        in_maps.append({
            "x": np.ascontiguousarray(xb[b]),
            "xb": np.ascontiguousarray(xb[b].astype(bf)),
            "wpack": wpack, "bpack": bpack,
        })
    return in_maps, rbound


def run(inputs, trace=False, **kw):
    from concourse.bass_utils import run_bass_kernel_spmd

    in_maps, rbound = host_prep(inputs)
    use_dve_exp = rbound <= 3.0  # poly accurate enough; else ACT-only exp
    fast_recip = True
    if _FORCE is not None:
        use_dve_exp, fast_recip = _FORCE
    key = ("nc", round(rbound, 3) if use_dve_exp else None, fast_recip)
    if key not in _CACHE:
        coef, _fit_err = fit_exp_poly(max(rbound, 1.0)) if use_dve_exp else (
            [1.0, 1.0, 1.0], 0.0)
        _CACHE.clear()
        _CACHE[key] = build_nc(coef, use_dve_exp=use_dve_exp,
                               fast_recip=fast_recip)
    nc = _CACHE[key]
    try:
        res = run_bass_kernel_spmd(nc, in_maps, core_ids=list(range(NCORES)),
                                   trace=trace, **kw)
    except Exception:
        # transient device wedge -- retry once
        res = run_bass_kernel_spmd(nc, in_maps, core_ids=list(range(NCORES)),
                                   trace=trace, **kw)
    y = np.stack([np.asarray(res.results[b]["y"]) for b in range(B)])
    y = y.reshape(B, C, W, H).astype(np.float32)
    return y, res


def kernel(**inputs) -> np.ndarray:
    y, _ = run(inputs)
    return y
